# revision 8
# baseline (speedup 1.0000x reference)
"""GNN message-passing aggregator on 8 Trainium2 NeuronCores.

Computes, for the full graph:
    node = entity_embed * out_sqrt_degree
    msg  = node[src] * edge_weight
    N_h  = segment_sum(msg, dst, N) * in_sqrt_degree
    out  = leaky_relu((entity_embed + N_h) @ W.T + b, 0.01)

Strategy (dst-partitioned edge shard, no collectives):
  * Host: fold degree factors into the gather table / edge weights, sort
    edges by dst, cut the node space into 392 aligned 128-node tiles,
    and deal tiles onto 8 cores x 49 slots so every core runs the same
    instruction stream (SPMD: one program, per-core data).
  * Device, per 128-node tile: dma_gather the source rows (256B each)
    from the replicated node table, build a weighted one-hot selection
    matrix S[e, n] = (dst_local[e] == n) * w[e] with one fused DVE
    tensor_scalar over a constant iota, and accumulate
    N_h.T += gathered.T @ S on the PE in PSUM.  The dst-sorted edge
    order makes each tile's edges contiguous, so no scatter and no
    cross-core reduction is needed.  The small linear runs per tile:
    out = leaky(X.T.T @ W.T + b) via two more matmuls (ones-row trick
    for the bias), leaky via max(x, 0.01x) on DVE.
  * src indices must fit int16 for dma_gather, so the 50176-row table is
    gathered as two 25088-row halves; each tile's edge list is split by
    src half (A: src < 25088, B: src >= 25088), each half padded to a
    multiple of 128 edges with (idx=0, w=0) no-op edges.
  * dma_gather tops out at 1024 indices per instruction, and each
    instruction's descriptor generation runs on one Q7 core pair chosen
    by queue_num.  So the per-core A/B edge streams are chunked into
    8-block (1024-edge) gathers independent of slot boundaries, cycled
    over 4 SWDGE queues to parallelize descriptor generation.
"""

import json
import sys
import types

import numpy as np

P = 128
D = 64
N_NODES = 50000
N_CORES = 8
HALF = 25088            # int16-safe gather-table half (196 * 128)
NPAD = 2 * HALF         # 50176 = 392 tiles
NT = NPAD // P          # 392
SLOTS = NT // N_CORES   # 49
CHUNK = 8               # blocks per dma_gather
NQ = 4                  # SWDGE queues (Q7 core pairs)


# ----------------------------------------------------------------------------
# Environment fixups (self-contained; kernel.py must run alone).
# ----------------------------------------------------------------------------

_SPLIT_COUNT = 0


def _split_multi_waits_json(bir: bytes) -> bytes:
    """This container's walrus accepts only ONE sync wait per instruction
    ('Too many sync wait commands'), while Tile's scheduler attaches
    several.  Rewrite each instruction with N>1 waits into N-1 same-engine
    NoOps (one wait each) followed by the instruction with the last wait;
    same-engine sequencer order makes this equivalent."""
    global _SPLIT_COUNT
    d = json.loads(bir)
    changed = False
    for fn in d.get("functions", []):
        for bb in fn.get("blocks", []):
            out = []
            for ins in bb.get("instructions", []):
                si = ins.get("sync_info") or {}
                ow = si.get("on_wait") or []
                if len(ow) > 1:
                    changed = True
                    for w in ow[:-1]:
                        _SPLIT_COUNT += 1
                        out.append({
                            "opcode": "NoOp",
                            "engine": ins.get("engine", "Unassigned"),
                            "name": f"I-waitsplit-{_SPLIT_COUNT}",
                            "ins": [],
                            "outs": [],
                            "sync_info": {"on_update": [], "on_wait": [w]},
                        })
                    si["on_wait"] = [ow[-1]]
                out.append(ins)
            bb["instructions"] = out
    return json.dumps(d).encode() if changed else bir


def _install_fixups():
    import concourse.bass_utils as bass_utils
    import concourse.bass2jax as bass2jax

    if not getattr(bass_utils, "_waitsplit_installed", False):
        bass_utils._waitsplit_installed = True
        orig_compile = bass_utils.compile_bir_kernel

        def patched_compile(bir_json, tmpdir, neff_name="file.neff"):
            if isinstance(bir_json, str):
                bir_json = bir_json.encode()
            return orig_compile(_split_multi_waits_json(bir_json), tmpdir,
                                neff_name=neff_name)

        bass_utils.compile_bir_kernel = patched_compile
        bass2jax.compile_bir_kernel = patched_compile
        # No artifact bucket in this container; keep profiles local.
        bass_utils.upload_artifacts = lambda tmpdir: tmpdir

    # run_bass_kernel_spmd(trace=True) under axon needs antenv.axon_hooks,
    # which this image doesn't ship.  Synthesize it and install the ctypes
    # NTFF hook from trn_agent_boot so neuron-profile works.
    if "antenv.axon_hooks" not in sys.modules:
        m = types.ModuleType("antenv.axon_hooks")
        m._hook = None
        m.set_axon_ntff_profile_hook = lambda h: setattr(m, "_hook", h)
        m.get_axon_ntff_profile_hook = lambda: m._hook
        sys.modules["antenv.axon_hooks"] = m
        try:
            import antenv
            antenv.axon_hooks = m
        except ImportError:
            pass
        try:
            from trn_agent_boot.trn_boot import _ntff_profile_via_ctypes
            hook = _ntff_profile_via_ctypes("/opt/axon/libaxon_pjrt.so")
            if hook is not None:
                m._hook = hook
        except Exception:
            pass


# ----------------------------------------------------------------------------
# Host-side graph partitioning
# ----------------------------------------------------------------------------

def _wrap16(seg: np.ndarray) -> np.ndarray:
    """dma_gather index layout: index i lives at [i % 16, i // 16]."""
    assert seg.size % 16 == 0
    return seg.reshape(-1, 16).T


def _prepare(entity_embed, src, dst, edge_weight, out_sqrt_degree,
             in_sqrt_degree):
    f32 = np.float32
    node = (entity_embed * out_sqrt_degree).astype(f32)
    node_pad = np.zeros((NPAD, D), f32)
    node_pad[:N_NODES] = node
    embT_pad = np.zeros((D, NPAD), f32)
    embT_pad[:, :N_NODES] = entity_embed.astype(f32).T

    ew2 = (edge_weight[:, 0] * in_sqrt_degree[dst, 0]).astype(f32)

    order = np.argsort(dst, kind="stable")
    sdst = dst[order].astype(np.int64)
    ssrc = src[order].astype(np.int64)
    sew = ew2[order]

    counts = np.bincount(sdst // P, minlength=NT)
    starts = np.concatenate([[0], np.cumsum(counts)])

    # Per tile: split by gather-table half, count padded 128-edge blocks.
    tiles = []
    for t in range(NT):
        lo, hi = starts[t], starts[t + 1]
        t_src = ssrc[lo:hi]
        t_dstl = (sdst[lo:hi] - t * P).astype(f32)
        t_ew = sew[lo:hi]
        a = t_src < HALF
        tiles.append((t, t_src[a], t_dstl[a], t_ew[a],
                      t_src[~a] - HALF, t_dstl[~a], t_ew[~a]))
    ba = np.array([-(-len(x[1]) // P) for x in tiles])
    bb = np.array([-(-len(x[4]) // P) for x in tiles])

    # Deal tiles into 49 slots x 8 cores; similar (BA, BB) tiles share a
    # slot so the per-slot max padding stays small.  Slot block counts are
    # shared by all cores (one SPMD program).
    rank = sorted(range(NT), key=lambda t: (-ba[t], -bb[t]))
    slot_ba = np.zeros(SLOTS, np.int64)
    slot_bb = np.zeros(SLOTS, np.int64)
    tile_of = np.zeros((N_CORES, SLOTS), np.int64)
    for s in range(SLOTS):
        octet = rank[s * N_CORES:(s + 1) * N_CORES]
        slot_ba[s] = max(ba[t] for t in octet)
        slot_bb[s] = max(bb[t] for t in octet)
        for c, t in enumerate(octet):
            tile_of[c, s] = t

    ta = int(slot_ba.sum())           # A-stream blocks per core
    tbb = int(slot_bb.sum())          # B-stream blocks per core
    tb = ta + tbb
    # idx columns padded so each stream's chunks are CHUNK-aligned
    CH = CHUNK
    acols = 8 * CH * (-(-ta // CH))
    bcols = 8 * CH * (-(-tbb // CH))
    icols = acols + bcols

    idx_all = np.zeros((N_CORES, 16, icols), np.int16)
    dstl_all = np.zeros((N_CORES, P, tb), f32)
    ew_all = np.zeros((N_CORES, P, tb), f32)
    embT_all = np.zeros((N_CORES, D, SLOTS * P), f32)

    a0 = np.concatenate([[0], np.cumsum(slot_ba)])   # A-stream block offsets
    b0 = np.concatenate([[0], np.cumsum(slot_bb)])   # B-stream block offsets

    for c in range(N_CORES):
        for s in range(SLOTS):
            t, srcA, dstlA, ewA, srcB, dstlB, ewB = tiles[tile_of[c, s]]
            nA, nB = P * slot_ba[s], P * slot_bb[s]
            ia = np.zeros(nA, np.int16)
            ia[:len(srcA)] = srcA
            ib = np.zeros(nB, np.int16)
            ib[:len(srcB)] = srcB
            # idx columns: A-stream first [0, acols), then B-stream.
            ca = 8 * a0[s]
            cb = acols + 8 * b0[s]
            if nA:
                idx_all[c, :, ca:ca + nA // 16] = _wrap16(ia)
            if nB:
                idx_all[c, :, cb:cb + nB // 16] = _wrap16(ib)
            # dstl/ew columns: A-block (s,j) -> a0[s]+j; B-block -> ta+b0[s]+j
            if slot_ba[s]:
                dl = np.zeros(nA, f32)
                dl[:len(dstlA)] = dstlA
                ws = np.zeros(nA, f32)
                ws[:len(ewA)] = ewA
                k = a0[s]
                dstl_all[c, :, k:k + slot_ba[s]] = dl.reshape(-1, P).T
                ew_all[c, :, k:k + slot_ba[s]] = ws.reshape(-1, P).T
            if slot_bb[s]:
                dl = np.zeros(nB, f32)
                dl[:len(dstlB)] = dstlB
                ws = np.zeros(nB, f32)
                ws[:len(ewB)] = ewB
                k = ta + b0[s]
                dstl_all[c, :, k:k + slot_bb[s]] = dl.reshape(-1, P).T
                ew_all[c, :, k:k + slot_bb[s]] = ws.reshape(-1, P).T
            embT_all[c, :, s * P:(s + 1) * P] = embT_pad[:, t * P:(t + 1) * P]

    idx_rep = np.tile(idx_all, (1, 8, 1))  # replicate across the 8 Q7 cores
    sig = (tuple(int(x) for x in slot_ba), tuple(int(x) for x in slot_bb))
    return (node_pad, idx_rep, dstl_all, ew_all, embT_all, tile_of, sig, tb,
            icols)


# ----------------------------------------------------------------------------
# Device program
# ----------------------------------------------------------------------------

_PROGRAM_CACHE = {}


class _GatherStream:
    """Lazily emits chunked (<=CHUNK blocks) dma_gathers over one table
    half's concatenated block stream.  Per chunk it also emits ONE batched
    edge-weight multiply (gm = g * ew) and ONE batched one-hot build
    (S[:, j, n] = (iota[n] == dstl[j])) so the DVE cost is amortized over
    CHUNK blocks.  block(i) returns (lhsT, rhs) APs for block i."""

    def __init__(self, nc, mybir, pool, gmpool, spool, table_ap, idx_segs,
                 blk_col0, total_blocks, dstl_sb, ew_sb, iota_rep,
                 qpick, f32):
        self.nc = nc
        self.mybir = mybir
        self.pool = pool
        self.gmpool = gmpool
        self.spool = spool
        self.table_ap = table_ap
        # idx_segs: list of (tile, chunk0, nchunks); chunk k's 8*CHUNK idx
        # columns live in its group's tile at offset (k - chunk0)*8*CHUNK.
        self.idx_segs = idx_segs
        self.blk_col0 = blk_col0   # column offset into dstl/ew for block 0
        self.total = total_blocks
        self.dstl_sb = dstl_sb
        self.ew_sb = ew_sb
        self.iota_rep = iota_rep
        self.qpick = qpick
        self.f32 = f32
        self.tiles = []            # chunk index -> (gm tile, S tile)

    def _idx_ap(self, k, cols):
        for t, c0, nch in self.idx_segs:
            if c0 <= k < c0 + nch:
                off = (k - c0) * 8 * CHUNK
                return t[:, off:off + cols]
        raise AssertionError(k)

    def _emit_chunk(self, k):
        nc = self.nc
        nblk = min(CHUNK, self.total - k * CHUNK)
        g = self.pool.tile([P, nblk, D], self.f32)
        n = P * nblk
        nc.gpsimd.dma_gather(
            g[:], self.table_ap, self._idx_ap(k, n // 16), n, n, D,
            queue_num=self.qpick(), single_packet=False)
        b0 = self.blk_col0 + CHUNK * k
        gm = self.gmpool.tile([P, nblk, D], self.f32)
        nc.vector.tensor_tensor(
            out=gm[:], in0=g[:],
            in1=self.ew_sb[:, b0:b0 + nblk].to_broadcast([P, nblk, D]),
            op=self.mybir.AluOpType.mult)
        S = self.spool.tile([P, nblk, P], self.f32)
        nc.vector.tensor_tensor(
            out=S[:],
            in0=self.iota_rep[:, :nblk * P].rearrange(
                "p (k n) -> p k n", n=P),
            in1=self.dstl_sb[:, b0:b0 + nblk].to_broadcast([P, nblk, P]),
            op=self.mybir.AluOpType.is_equal)
        self.tiles.append((gm, S))

    def block(self, i):
        k, off = divmod(i, CHUNK)
        while len(self.tiles) <= k:
            self._emit_chunk(len(self.tiles))
        gm, S = self.tiles[k]
        return gm[:, off, :], S[:, off, :]


def _build_program(sig, tb, icols):
    if sig in _PROGRAM_CACHE:
        return _PROGRAM_CACHE[sig]

    from concourse import bacc
    import concourse.mybir as mybir
    import concourse.tile as tile

    slot_ba, slot_bb = sig
    ta = sum(slot_ba)
    tbb = sum(slot_bb)
    nc = bacc.Bacc("TRN2", num_swdge_queues=NQ)
    f32 = mybir.dt.float32
    t_node = nc.dram_tensor("node", [NPAD, D], f32, kind="ExternalInput")
    t_idx = nc.dram_tensor("idx", [P, icols], mybir.dt.int16,
                           kind="ExternalInput")
    t_dstl = nc.dram_tensor("dstl", [P, tb], f32, kind="ExternalInput")
    t_ew = nc.dram_tensor("ew", [P, tb], f32, kind="ExternalInput")
    t_embT = nc.dram_tensor("embT", [D, SLOTS * P], f32, kind="ExternalInput")
    t_wt = nc.dram_tensor("wt", [D, D], f32, kind="ExternalInput")
    t_b = nc.dram_tensor("bias", [1, D], f32, kind="ExternalInput")
    t_iota = nc.dram_tensor("iota", [P, CHUNK * P], f32,
                            kind="ExternalInput")
    t_out = nc.dram_tensor("out", [SLOTS * P, D], f32, kind="ExternalOutput")

    qstate = [0]

    def qpick():
        q = qstate[0] % NQ
        qstate[0] += 1
        return q

    with tile.TileContext(nc) as tc:
        with tc.tile_pool(name="const", bufs=1) as cpool, \
             tc.tile_pool(name="ga", bufs=4) as gapool, \
             tc.tile_pool(name="gb", bufs=4) as gbpool, \
             tc.tile_pool(name="gma", bufs=3) as gmapool, \
             tc.tile_pool(name="gmb", bufs=3) as gmbpool, \
             tc.tile_pool(name="sa", bufs=3) as sapool, \
             tc.tile_pool(name="sb", bufs=3) as sbpool, \
             tc.tile_pool(name="small", bufs=3) as mpool, \
             tc.tile_pool(name="psnh", bufs=3, space="PSUM") as psnh, \
             tc.tile_pool(name="psout", bufs=2, space="PSUM") as psout:
            # idx group tiles (chunk-aligned) so the first gather only
            # waits on its own small DMA, not the whole index array
            ncha = -(-ta // CHUNK)
            nchb = -(-tbb // CHUNK)
            segs = []
            for c0t, ncht in ((0, ncha), (ncha, nchb)):
                ngrp = min(4, ncht) or 1
                for gidx in range(ngrp):
                    lo = c0t + ncht * gidx // ngrp
                    hi = c0t + ncht * (gidx + 1) // ngrp
                    if hi == lo:
                        continue
                    w = min(hi * 8 * CHUNK, icols // 1) - lo * 8 * CHUNK
                    w = min(w, icols - lo * 8 * CHUNK)
                    tgt = cpool.tile([P, w], mybir.dt.int16,
                                     tag=f"idx{lo}")
                    nc.sync.dma_start(
                        out=tgt[:],
                        in_=t_idx[:, lo * 8 * CHUNK:lo * 8 * CHUNK + w])
                    segs.append((tgt, lo, hi - lo))
            idx_segs_a = [(t, c0, n) for (t, c0, n) in segs if c0 < ncha]
            idx_segs_b = [(t, c0 - ncha, n) for (t, c0, n) in segs
                          if c0 >= ncha]
            dstl_sb = cpool.tile([P, tb], f32)
            ew_sb = cpool.tile([P, tb], f32)
            bnd2 = [tb * i // 2 for i in range(3)]
            for i in range(2):
                nc.scalar.dma_start(out=dstl_sb[:, bnd2[i]:bnd2[i + 1]],
                                    in_=t_dstl[:, bnd2[i]:bnd2[i + 1]])
                nc.scalar.dma_start(out=ew_sb[:, bnd2[i]:bnd2[i + 1]],
                                    in_=t_ew[:, bnd2[i]:bnd2[i + 1]])
            iota_rep = cpool.tile([P, CHUNK * P], f32)
            nc.scalar.dma_start(out=iota_rep[:], in_=t_iota[:])
            ones = cpool.tile([1, P], f32)
            nc.vector.memset(ones[:], 1.0)
            wt_sb = cpool.tile([D, D], f32)
            nc.scalar.dma_start(out=wt_sb[:], in_=t_wt[:])
            b_sb = cpool.tile([1, D], f32)
            nc.scalar.dma_start(out=b_sb[:], in_=t_b[:])
            embT_sb = cpool.tile([D, SLOTS * P], f32)
            embT_loaded = [False]

            def load_embT():
                if not embT_loaded[0]:
                    embT_loaded[0] = True
                    half = SLOTS * P // 2
                    nc.scalar.dma_start(out=embT_sb[:, :half],
                                        in_=t_embT[:, :half])
                    nc.scalar.dma_start(out=embT_sb[:, half:],
                                        in_=t_embT[:, half:])

            sa = _GatherStream(nc, mybir, gapool, gmapool, sapool,
                               t_node[0:HALF, :], idx_segs_a, 0, ta,
                               dstl_sb, ew_sb, iota_rep, qpick, f32)
            sb = _GatherStream(nc, mybir, gbpool, gmbpool, sbpool,
                               t_node[HALF:NPAD, :], idx_segs_b, ta, tbb,
                               dstl_sb, ew_sb, iota_rep, qpick, f32)

            a_off = 0
            b_off = 0
            for s in range(SLOTS):
                ba, bb = slot_ba[s], slot_bb[s]
                nb = ba + bb
                blocks = [sa.block(a_off + j) for j in range(ba)]
                blocks += [sb.block(b_off + j) for j in range(bb)]
                load_embT()
                a_off += ba
                b_off += bb
                xT = mpool.tile([D, P], f32, tag="xT")
                if nb:
                    nh = psnh.tile([D, P], f32, space="PSUM", tag="nh")
                    for i, (lhsT, rhs) in enumerate(blocks):
                        nc.tensor.matmul(out=nh[:], lhsT=lhsT, rhs=rhs,
                                         start=(i == 0), stop=(i == nb - 1))
                    nc.vector.tensor_add(out=xT[:], in0=nh[:],
                                         in1=embT_sb[:, s * P:(s + 1) * P])
                else:
                    nc.vector.tensor_copy(out=xT[:],
                                          in_=embT_sb[:, s * P:(s + 1) * P])
                o_ps = psout.tile([P, D], f32, space="PSUM", tag="ops")
                nc.tensor.matmul(out=o_ps[:], lhsT=xT[:], rhs=wt_sb[:],
                                 start=True, stop=False)
                nc.tensor.matmul(out=o_ps[:], lhsT=ones[:], rhs=b_sb[:],
                                 start=False, stop=True)
                o_scaled = mpool.tile([P, D], f32, tag="osc")
                nc.vector.tensor_scalar_mul(o_scaled[:], o_ps[:], 0.01)
                o_sb = mpool.tile([P, D], f32, tag="osb")
                nc.vector.tensor_tensor(out=o_sb[:], in0=o_ps[:],
                                        in1=o_scaled[:],
                                        op=mybir.AluOpType.max)
                nc.sync.dma_start(out=t_out[s * P:(s + 1) * P, :], in_=o_sb[:])

    nc.compile()
    _PROGRAM_CACHE[sig] = nc
    return nc


LAST_RESULTS = None


def kernel(entity_embed, src, dst, edge_weight, out_sqrt_degree,
           in_sqrt_degree, W, b):
    _install_fixups()
    from concourse.bass_utils import run_bass_kernel_spmd

    entity_embed = np.asarray(entity_embed, np.float32)
    src = np.asarray(src)
    dst = np.asarray(dst)
    edge_weight = np.asarray(edge_weight, np.float32)
    out_sqrt_degree = np.asarray(out_sqrt_degree, np.float32)
    in_sqrt_degree = np.asarray(in_sqrt_degree, np.float32)
    W = np.asarray(W, np.float32)
    b = np.asarray(b, np.float32)

    (node_pad, idx_rep, dstl_all, ew_all, embT_all, tile_of, sig, tb,
     icols) = _prepare(entity_embed, src, dst, edge_weight, out_sqrt_degree,
                       in_sqrt_degree)

    nc = _build_program(sig, tb, icols)

    wt = np.ascontiguousarray(W.T)          # rhs[k, j] = W[j, k]
    iota_np = np.tile(np.tile(np.arange(P, dtype=np.float32), CHUNK), (P, 1))
    in_maps = []
    for c in range(N_CORES):
        in_maps.append({
            "node": node_pad,
            "idx": np.ascontiguousarray(idx_rep[c]),
            "dstl": np.ascontiguousarray(dstl_all[c]),
            "ew": np.ascontiguousarray(ew_all[c]),
            "embT": np.ascontiguousarray(embT_all[c]),
            "wt": wt,
            "bias": b[None, :],
            "iota": iota_np,
        })

    res = run_bass_kernel_spmd(nc, in_maps, core_ids=list(range(N_CORES)))
    global LAST_RESULTS
    LAST_RESULTS = res

    out = np.empty((NPAD, D), np.float32)
    for c in range(N_CORES):
        oc = res.results[c]["out"]
        for s in range(SLOTS):
            t = tile_of[c, s]
            out[t * P:(t + 1) * P] = oc[s * P:(s + 1) * P]
    return out[:N_NODES]


# revision 10
# speedup vs baseline: 1.0305x; 1.0305x over previous
"""GNN message-passing aggregator on 8 Trainium2 NeuronCores.

Computes, for the full graph:
    node = entity_embed * out_sqrt_degree
    msg  = node[src] * edge_weight
    N_h  = segment_sum(msg, dst, N) * in_sqrt_degree
    out  = leaky_relu((entity_embed + N_h) @ W.T + b, 0.01)

Strategy (dst-partitioned edge shard, no collectives):
  * Host: fold degree factors into the gather table / edge weights, sort
    edges by dst, cut the node space into 392 aligned 128-node tiles,
    and deal tiles onto 8 cores x 49 slots so every core runs the same
    instruction stream (SPMD: one program, per-core data).
  * Device, per 128-node tile: dma_gather the source rows (256B each)
    from the replicated node table, build a one-hot selection matrix
    S[e, n] = (dst_local[e] == n) against a constant iota, weight the
    gathered rows by edge weight, and accumulate N_h.T += gm.T @ S on
    the PE in PSUM.  The dst-sorted edge order makes each tile's edges
    contiguous, so no scatter and no cross-core reduction is needed.
    The small linear runs per tile: out = leaky(X @ W.T + b) via two
    more matmuls (ones-row trick for the bias), leaky = max(x, 0.01x).
  * src indices must fit int16 for dma_gather, so the 50176-row table is
    gathered as two 25088-row halves; each tile's edge list is split by
    src half (A: src < 25088, B: src >= 25088), each half padded to a
    multiple of 128 edges with (idx=0, w=0) no-op edges.
  * dma_gather descriptor generation runs on one Q7 core pair selected
    by queue_num (~9.5ns/idx on the pair) — the whole kernel is bound by
    it.  The per-core A/B edge streams are chunked into 8-block
    (1024-edge) gathers independent of slot boundaries and cycled over
    4 SWDGE queues so up to four generators run concurrently
    (single_packet=False lifts the 64-descriptor packet cap).
  * DVE per-op fixed costs dominate per-block elementwise work, so the
    one-hot build and the edge-weight multiply are batched per chunk as
    single tensor_tensor ops over [128, nblk*128] / [128, nblk*64] using
    0-stride broadcast APs of the per-block scalars.
"""

import json
import sys
import types

import numpy as np

P = 128
D = 64
N_NODES = 50000
N_CORES = 8
HALF = 25088            # int16-safe gather-table half (196 * 128)
NPAD = 2 * HALF         # 50176 = 392 tiles
NT = NPAD // P          # 392
SLOTS = NT // N_CORES   # 49
CHUNK = 8               # blocks per dma_gather
NQ = 4                  # SWDGE queues (Q7 core pairs)


# ----------------------------------------------------------------------------
# Environment fixups (self-contained; kernel.py must run alone).
# ----------------------------------------------------------------------------

_SPLIT_COUNT = 0


def _split_multi_waits_json(bir: bytes) -> bytes:
    """This container's walrus accepts only ONE sync wait per instruction
    ('Too many sync wait commands'), while Tile's scheduler attaches
    several.  Rewrite each instruction with N>1 waits into N-1 same-engine
    NoOps (one wait each) followed by the instruction with the last wait;
    same-engine sequencer order makes this equivalent."""
    global _SPLIT_COUNT
    d = json.loads(bir)
    changed = False
    for fn in d.get("functions", []):
        for bb in fn.get("blocks", []):
            out = []
            for ins in bb.get("instructions", []):
                si = ins.get("sync_info") or {}
                ow = si.get("on_wait") or []
                if len(ow) > 1:
                    changed = True
                    for w in ow[:-1]:
                        _SPLIT_COUNT += 1
                        out.append({
                            "opcode": "NoOp",
                            "engine": ins.get("engine", "Unassigned"),
                            "name": f"I-waitsplit-{_SPLIT_COUNT}",
                            "ins": [],
                            "outs": [],
                            "sync_info": {"on_update": [], "on_wait": [w]},
                        })
                    si["on_wait"] = [ow[-1]]
                out.append(ins)
            bb["instructions"] = out
    return json.dumps(d).encode() if changed else bir


def _install_fixups():
    import concourse.bass_utils as bass_utils
    import concourse.bass2jax as bass2jax

    if not getattr(bass_utils, "_waitsplit_installed", False):
        bass_utils._waitsplit_installed = True
        orig_compile = bass_utils.compile_bir_kernel

        def patched_compile(bir_json, tmpdir, neff_name="file.neff"):
            if isinstance(bir_json, str):
                bir_json = bir_json.encode()
            return orig_compile(_split_multi_waits_json(bir_json), tmpdir,
                                neff_name=neff_name)

        bass_utils.compile_bir_kernel = patched_compile
        bass2jax.compile_bir_kernel = patched_compile
        # No artifact bucket in this container; keep profiles local.
        bass_utils.upload_artifacts = lambda tmpdir: tmpdir

    # run_bass_kernel_spmd(trace=True) under axon needs antenv.axon_hooks,
    # which this image doesn't ship.  Synthesize it and install the ctypes
    # NTFF hook from trn_agent_boot so neuron-profile works.
    if "antenv.axon_hooks" not in sys.modules:
        m = types.ModuleType("antenv.axon_hooks")
        m._hook = None
        m.set_axon_ntff_profile_hook = lambda h: setattr(m, "_hook", h)
        m.get_axon_ntff_profile_hook = lambda: m._hook
        sys.modules["antenv.axon_hooks"] = m
        try:
            import antenv
            antenv.axon_hooks = m
        except ImportError:
            pass
        try:
            from trn_agent_boot.trn_boot import _ntff_profile_via_ctypes
            hook = _ntff_profile_via_ctypes("/opt/axon/libaxon_pjrt.so")
            if hook is not None:
                m._hook = hook
        except Exception:
            pass


# ----------------------------------------------------------------------------
# Host-side graph partitioning
# ----------------------------------------------------------------------------

def _wrap16(seg: np.ndarray) -> np.ndarray:
    """dma_gather index layout: index i lives at [i % 16, i // 16]."""
    assert seg.size % 16 == 0
    return seg.reshape(-1, 16).T


def _prepare(entity_embed, src, dst, edge_weight, out_sqrt_degree,
             in_sqrt_degree):
    f32 = np.float32
    node = (entity_embed * out_sqrt_degree).astype(f32)
    node_pad = np.zeros((NPAD, D), f32)
    node_pad[:N_NODES] = node
    embT_pad = np.zeros((D, NPAD), f32)
    embT_pad[:, :N_NODES] = entity_embed.astype(f32).T

    ew2 = (edge_weight[:, 0] * in_sqrt_degree[dst, 0]).astype(f32)

    order = np.argsort(dst, kind="stable")
    sdst = dst[order].astype(np.int64)
    ssrc = src[order].astype(np.int64)
    sew = ew2[order]

    counts = np.bincount(sdst // P, minlength=NT)
    starts = np.concatenate([[0], np.cumsum(counts)])

    # Per tile: split by gather-table half, count padded 128-edge blocks.
    tiles = []
    for t in range(NT):
        lo, hi = starts[t], starts[t + 1]
        t_src = ssrc[lo:hi]
        t_dstl = (sdst[lo:hi] - t * P).astype(f32)
        t_ew = sew[lo:hi]
        a = t_src < HALF
        tiles.append((t, t_src[a], t_dstl[a], t_ew[a],
                      t_src[~a] - HALF, t_dstl[~a], t_ew[~a]))
    ba = np.array([-(-len(x[1]) // P) for x in tiles])
    bb = np.array([-(-len(x[4]) // P) for x in tiles])

    # Deal tiles into 49 slots x 8 cores; similar (BA, BB) tiles share a
    # slot so the per-slot max padding stays small.  Slot block counts are
    # shared by all cores (one SPMD program).
    rank = sorted(range(NT), key=lambda t: (-ba[t], -bb[t]))
    slot_ba = np.zeros(SLOTS, np.int64)
    slot_bb = np.zeros(SLOTS, np.int64)
    tile_of = np.zeros((N_CORES, SLOTS), np.int64)
    for s in range(SLOTS):
        octet = rank[s * N_CORES:(s + 1) * N_CORES]
        slot_ba[s] = max(ba[t] for t in octet)
        slot_bb[s] = max(bb[t] for t in octet)
        for c, t in enumerate(octet):
            tile_of[c, s] = t

    ta = int(slot_ba.sum())           # A-stream blocks per core
    tbb = int(slot_bb.sum())          # B-stream blocks per core
    tb = ta + tbb
    # idx columns padded so each stream's chunks are CHUNK-aligned
    CH = CHUNK
    acols = 8 * CH * (-(-ta // CH))
    bcols = 8 * CH * (-(-tbb // CH))
    icols = acols + bcols

    idx_all = np.zeros((N_CORES, 16, icols), np.int16)
    dstl_all = np.zeros((N_CORES, P, tb), f32)
    ew_all = np.zeros((N_CORES, P, tb), f32)
    embT_all = np.zeros((N_CORES, D, SLOTS * P), f32)

    a0 = np.concatenate([[0], np.cumsum(slot_ba)])   # A-stream block offsets
    b0 = np.concatenate([[0], np.cumsum(slot_bb)])   # B-stream block offsets

    for c in range(N_CORES):
        for s in range(SLOTS):
            t, srcA, dstlA, ewA, srcB, dstlB, ewB = tiles[tile_of[c, s]]
            nA, nB = P * slot_ba[s], P * slot_bb[s]
            ia = np.zeros(nA, np.int16)
            ia[:len(srcA)] = srcA
            ib = np.zeros(nB, np.int16)
            ib[:len(srcB)] = srcB
            # idx columns: A-stream first [0, acols), then B-stream.
            ca = 8 * a0[s]
            cb = acols + 8 * b0[s]
            if nA:
                idx_all[c, :, ca:ca + nA // 16] = _wrap16(ia)
            if nB:
                idx_all[c, :, cb:cb + nB // 16] = _wrap16(ib)
            # dstl/ew columns: A-block (s,j) -> a0[s]+j; B-block -> ta+b0[s]+j
            if slot_ba[s]:
                dl = np.zeros(nA, f32)
                dl[:len(dstlA)] = dstlA
                ws = np.zeros(nA, f32)
                ws[:len(ewA)] = ewA
                k = a0[s]
                dstl_all[c, :, k:k + slot_ba[s]] = dl.reshape(-1, P).T
                ew_all[c, :, k:k + slot_ba[s]] = ws.reshape(-1, P).T
            if slot_bb[s]:
                dl = np.zeros(nB, f32)
                dl[:len(dstlB)] = dstlB
                ws = np.zeros(nB, f32)
                ws[:len(ewB)] = ewB
                k = ta + b0[s]
                dstl_all[c, :, k:k + slot_bb[s]] = dl.reshape(-1, P).T
                ew_all[c, :, k:k + slot_bb[s]] = ws.reshape(-1, P).T
            embT_all[c, :, s * P:(s + 1) * P] = embT_pad[:, t * P:(t + 1) * P]

    idx_rep = np.tile(idx_all, (1, 8, 1))  # replicate across the 8 Q7 cores
    sig = (tuple(int(x) for x in slot_ba), tuple(int(x) for x in slot_bb))
    return (node_pad, idx_rep, dstl_all, ew_all, embT_all, tile_of, sig, tb,
            icols)


# ----------------------------------------------------------------------------
# Device program
# ----------------------------------------------------------------------------

_PROGRAM_CACHE = {}


class _GatherStream:
    """Lazily emits chunked (<=CHUNK blocks) dma_gathers over one table
    half's concatenated block stream.  Per chunk it also emits ONE batched
    edge-weight multiply (gm = g * ew) and ONE batched one-hot build
    (S[:, j, n] = (iota[n] == dstl[j])) so the DVE cost is amortized over
    CHUNK blocks.  block(i) returns (lhsT, rhs) APs for block i."""

    def __init__(self, nc, mybir, pool, gmpool, spool, table_ap, idx_segs,
                 blk_col0, total_blocks, dstl_sb, ew_sb, iota_rep,
                 qpick, f32):
        self.nc = nc
        self.mybir = mybir
        self.pool = pool
        self.gmpool = gmpool
        self.spool = spool
        self.table_ap = table_ap
        # idx_segs: list of (tile, chunk0, nchunks); chunk k's 8*CHUNK idx
        # columns live in its group's tile at offset (k - chunk0)*8*CHUNK.
        self.idx_segs = idx_segs
        self.blk_col0 = blk_col0   # column offset into dstl/ew for block 0
        self.total = total_blocks
        self.dstl_sb = dstl_sb
        self.ew_sb = ew_sb
        self.iota_rep = iota_rep
        self.qpick = qpick
        self.f32 = f32
        self.tiles = []            # chunk index -> (gm tile, S tile)

    def _idx_ap(self, k, cols):
        for t, c0, nch in self.idx_segs:
            if c0 <= k < c0 + nch:
                off = (k - c0) * 8 * CHUNK
                return t[:, off:off + cols]
        raise AssertionError(k)

    def _emit_chunk(self, k):
        nc = self.nc
        nblk = min(CHUNK, self.total - k * CHUNK)
        g = self.pool.tile([P, nblk, D], self.f32)
        n = P * nblk
        nc.gpsimd.dma_gather(
            g[:], self.table_ap, self._idx_ap(k, n // 16), n, n, D,
            queue_num=self.qpick(), single_packet=False)
        b0 = self.blk_col0 + CHUNK * k
        gm = self.gmpool.tile([P, nblk, D], self.f32)
        nc.vector.tensor_tensor(
            out=gm[:], in0=g[:],
            in1=self.ew_sb[:, b0:b0 + nblk].to_broadcast([P, nblk, D]),
            op=self.mybir.AluOpType.mult)
        S = self.spool.tile([P, nblk, P], self.f32)
        nc.vector.tensor_tensor(
            out=S[:],
            in0=self.iota_rep[:, :nblk * P].rearrange(
                "p (k n) -> p k n", n=P),
            in1=self.dstl_sb[:, b0:b0 + nblk].to_broadcast([P, nblk, P]),
            op=self.mybir.AluOpType.is_equal)
        self.tiles.append((gm, S))

    def block(self, i):
        k, off = divmod(i, CHUNK)
        while len(self.tiles) <= k:
            self._emit_chunk(len(self.tiles))
        gm, S = self.tiles[k]
        return gm[:, off, :], S[:, off, :]


def _build_program(sig, tb, icols):
    if sig in _PROGRAM_CACHE:
        return _PROGRAM_CACHE[sig]

    from concourse import bacc
    import concourse.mybir as mybir
    import concourse.tile as tile

    slot_ba, slot_bb = sig
    ta = sum(slot_ba)
    tbb = sum(slot_bb)
    nc = bacc.Bacc("TRN2", num_swdge_queues=NQ)
    f32 = mybir.dt.float32
    t_node = nc.dram_tensor("node", [NPAD, D], f32, kind="ExternalInput")
    t_idx = nc.dram_tensor("idx", [P, icols], mybir.dt.int16,
                           kind="ExternalInput")
    t_dstl = nc.dram_tensor("dstl", [P, tb], f32, kind="ExternalInput")
    t_ew = nc.dram_tensor("ew", [P, tb], f32, kind="ExternalInput")
    t_embT = nc.dram_tensor("embT", [D, SLOTS * P], f32, kind="ExternalInput")
    t_wt = nc.dram_tensor("wt", [D, D], f32, kind="ExternalInput")
    t_b = nc.dram_tensor("bias", [1, D], f32, kind="ExternalInput")
    t_iota = nc.dram_tensor("iota", [P, CHUNK * P], f32,
                            kind="ExternalInput")
    t_out = nc.dram_tensor("out", [SLOTS * P, D], f32, kind="ExternalOutput")

    qstate = [0]

    def qpick():
        q = qstate[0] % NQ
        qstate[0] += 1
        return q

    with tile.TileContext(nc) as tc:
        with tc.tile_pool(name="const", bufs=1) as cpool, \
             tc.tile_pool(name="ga", bufs=4) as gapool, \
             tc.tile_pool(name="gb", bufs=4) as gbpool, \
             tc.tile_pool(name="gma", bufs=3) as gmapool, \
             tc.tile_pool(name="gmb", bufs=3) as gmbpool, \
             tc.tile_pool(name="sa", bufs=3) as sapool, \
             tc.tile_pool(name="sb", bufs=3) as sbpool, \
             tc.tile_pool(name="small", bufs=3) as mpool, \
             tc.tile_pool(name="psnh", bufs=3, space="PSUM") as psnh, \
             tc.tile_pool(name="psout", bufs=2, space="PSUM") as psout:
            # idx group tiles (chunk-aligned) so the first gather only
            # waits on its own small DMA, not the whole index array
            ncha = -(-ta // CHUNK)
            nchb = -(-tbb // CHUNK)
            segs = []
            for c0t, ncht in ((0, ncha), (ncha, nchb)):
                ngrp = min(4, ncht) or 1
                for gidx in range(ngrp):
                    lo = c0t + ncht * gidx // ngrp
                    hi = c0t + ncht * (gidx + 1) // ngrp
                    if hi == lo:
                        continue
                    w = min(hi * 8 * CHUNK, icols // 1) - lo * 8 * CHUNK
                    w = min(w, icols - lo * 8 * CHUNK)
                    tgt = cpool.tile([P, w], mybir.dt.int16,
                                     tag=f"idx{lo}")
                    nc.sync.dma_start(
                        out=tgt[:],
                        in_=t_idx[:, lo * 8 * CHUNK:lo * 8 * CHUNK + w])
                    segs.append((tgt, lo, hi - lo))
            idx_segs_a = [(t, c0, n) for (t, c0, n) in segs if c0 < ncha]
            idx_segs_b = [(t, c0 - ncha, n) for (t, c0, n) in segs
                          if c0 >= ncha]
            dstl_sb = cpool.tile([P, tb], f32)
            ew_sb = cpool.tile([P, tb], f32)
            bnd2 = [tb * i // 4 for i in range(5)]
            for i in range(4):
                nc.sync.dma_start(out=dstl_sb[:, bnd2[i]:bnd2[i + 1]],
                                  in_=t_dstl[:, bnd2[i]:bnd2[i + 1]])
                nc.sync.dma_start(out=ew_sb[:, bnd2[i]:bnd2[i + 1]],
                                  in_=t_ew[:, bnd2[i]:bnd2[i + 1]])
            iota_rep = cpool.tile([P, CHUNK * P], f32)
            nc.scalar.dma_start(out=iota_rep[:], in_=t_iota[:])
            ones = cpool.tile([1, P], f32)
            nc.vector.memset(ones[:], 1.0)
            wt_sb = cpool.tile([D, D], f32)
            nc.scalar.dma_start(out=wt_sb[:], in_=t_wt[:])
            b_sb = cpool.tile([1, D], f32)
            nc.scalar.dma_start(out=b_sb[:], in_=t_b[:])
            embT_sb = cpool.tile([D, SLOTS * P], f32)
            nc.scalar.dma_start(out=embT_sb[:], in_=t_embT[:])

            sa = _GatherStream(nc, mybir, gapool, gmapool, sapool,
                               t_node[0:HALF, :], idx_segs_a, 0, ta,
                               dstl_sb, ew_sb, iota_rep, qpick, f32)
            sb = _GatherStream(nc, mybir, gbpool, gmbpool, sbpool,
                               t_node[HALF:NPAD, :], idx_segs_b, ta, tbb,
                               dstl_sb, ew_sb, iota_rep, qpick, f32)

            a_off = 0
            b_off = 0
            for s in range(SLOTS):
                ba, bb = slot_ba[s], slot_bb[s]
                nb = ba + bb
                blocks = [sa.block(a_off + j) for j in range(ba)]
                blocks += [sb.block(b_off + j) for j in range(bb)]
                a_off += ba
                b_off += bb
                xT = mpool.tile([D, P], f32, tag="xT")
                if nb:
                    nh = psnh.tile([D, P], f32, space="PSUM", tag="nh")
                    for i, (lhsT, rhs) in enumerate(blocks):
                        nc.tensor.matmul(out=nh[:], lhsT=lhsT, rhs=rhs,
                                         start=(i == 0), stop=(i == nb - 1))
                    nc.vector.tensor_add(out=xT[:], in0=nh[:],
                                         in1=embT_sb[:, s * P:(s + 1) * P])
                else:
                    nc.vector.tensor_copy(out=xT[:],
                                          in_=embT_sb[:, s * P:(s + 1) * P])
                o_ps = psout.tile([P, D], f32, space="PSUM", tag="ops")
                nc.tensor.matmul(out=o_ps[:], lhsT=xT[:], rhs=wt_sb[:],
                                 start=True, stop=False)
                nc.tensor.matmul(out=o_ps[:], lhsT=ones[:], rhs=b_sb[:],
                                 start=False, stop=True)
                o_scaled = mpool.tile([P, D], f32, tag="osc")
                nc.vector.tensor_scalar_mul(o_scaled[:], o_ps[:], 0.01)
                o_sb = mpool.tile([P, D], f32, tag="osb")
                nc.vector.tensor_tensor(out=o_sb[:], in0=o_ps[:],
                                        in1=o_scaled[:],
                                        op=mybir.AluOpType.max)
                nc.sync.dma_start(out=t_out[s * P:(s + 1) * P, :], in_=o_sb[:])

    nc.compile()
    _PROGRAM_CACHE[sig] = nc
    return nc


LAST_RESULTS = None


def kernel(entity_embed, src, dst, edge_weight, out_sqrt_degree,
           in_sqrt_degree, W, b):
    _install_fixups()
    from concourse.bass_utils import run_bass_kernel_spmd

    entity_embed = np.asarray(entity_embed, np.float32)
    src = np.asarray(src)
    dst = np.asarray(dst)
    edge_weight = np.asarray(edge_weight, np.float32)
    out_sqrt_degree = np.asarray(out_sqrt_degree, np.float32)
    in_sqrt_degree = np.asarray(in_sqrt_degree, np.float32)
    W = np.asarray(W, np.float32)
    b = np.asarray(b, np.float32)

    (node_pad, idx_rep, dstl_all, ew_all, embT_all, tile_of, sig, tb,
     icols) = _prepare(entity_embed, src, dst, edge_weight, out_sqrt_degree,
                       in_sqrt_degree)

    nc = _build_program(sig, tb, icols)

    wt = np.ascontiguousarray(W.T)          # rhs[k, j] = W[j, k]
    iota_np = np.tile(np.tile(np.arange(P, dtype=np.float32), CHUNK), (P, 1))
    in_maps = []
    for c in range(N_CORES):
        in_maps.append({
            "node": node_pad,
            "idx": np.ascontiguousarray(idx_rep[c]),
            "dstl": np.ascontiguousarray(dstl_all[c]),
            "ew": np.ascontiguousarray(ew_all[c]),
            "embT": np.ascontiguousarray(embT_all[c]),
            "wt": wt,
            "bias": b[None, :],
            "iota": iota_np,
        })

    res = run_bass_kernel_spmd(nc, in_maps, core_ids=list(range(N_CORES)))
    global LAST_RESULTS
    LAST_RESULTS = res

    out = np.empty((NPAD, D), np.float32)
    for c in range(N_CORES):
        oc = res.results[c]["out"]
        for s in range(SLOTS):
            t = tile_of[c, s]
            out[t * P:(t + 1) * P] = oc[s * P:(s + 1) * P]
    return out[:N_NODES]
